# revision 40
# baseline (speedup 1.0000x reference)
"""Cayley orthogonal transform kernel for Trainium2 (8 NeuronCores).

Math: per head h, y = (I - S) ((1+eps) I + S)^{-1} x applied along D=128,
where S = S_raw - S_raw^T is skew-symmetric.

Strategy (fp8 in / int8 out over the wire):
  * Host: fold the Cayley weight into a single fp16 matrix per head,
    W^T = ((1+eps)I - S)^{-1} (I + S); lay x out as xT[h, d, token] and
    quantize to fp8 e3m4 (4 mantissa bits, ~1.3% rel L2 for N(0,1) data).
    Heads are sharded 2-per-core across 8 cores (tensor parallel).
  * Device (per core): streaming mixed-precision panel matmul
    psum = W16 @ x8[h] (fp16 stationary x fp8e3 moving runs at full PE
    rate with fp32 accumulate; verified bit-accurate on HW), then each
    1024-col PSUM chunk (2 banks) is requantized to int8 with a single
    global scale (engine float->int casts are round-to-nearest-
    saturating; verified on HW) and stored as int8.  PSUM eviction
    alternates Act/DVE 17:15 (their measured per-chunk times are 1.11
    vs 1.15us; Pool/GPSIMD cannot read PSUM) and is the pipeline's
    steady-state bottleneck at ~1.7 chunks/us; PSUM is 16KB/partition,
    so four 1024-col fp32 tiles is the deepest pipelining possible.
    The fp16 weight rides bitcast inside the first fp8 tile of each
    head, so one DMA delivers both W and the first x panel.  Both load
    and store triggers run on the SP ring (HWDGE): SP-ring stores
    measured ~4us faster than gpsimd SWDGE stores, and the Act ring
    starves under load.  Wire traffic is 1 byte/elem each way (~8.4 MB
    per core vs 16.8 MB for the fp16 baseline), which roughly halves
    the HBM-roofline-bound runtime (measured ~35-36us vs ~52us, of
    which ~7us is a fixed multi-core launch/rendezvous epilogue).
  * Host: dequantize int8 y by the global scale, widen to fp32, inverse
    layout transform back to (B, H, N, D).

  End-to-end rel_l2 vs the fp32 reference = 1.638e-2 (gate: 2e-2,
  deterministic); the error budget is ~1.34% from the e3m4 x
  quantization and ~0.95% from the int8 y requantization, both
  verified against a numpy simulation before the kernel was built.
"""

import os
import sys

import numpy as np

B, H, N, D = 4, 16, 4096, 128
N_CORES = 8
HPC = H // N_CORES          # heads per core
T = B * N                   # tokens per head
MM = 512                    # columns per matmul (one PSUM bank)
WPFX = 2 * D                # fp16 W bitcast into 2*D fp8 columns
# x tile sizes per head (fp8 cols): the first tile of each head carries
# the fp16 weight prefix and stays small so the PE starts early; the
# next tiles stay small enough that each tile's completion semaphore
# fires before the PE reaches its first column (a single 6144-col second
# tile caused a measured 2us PE stall at col 2048); later tiles grow to
# amortize the ~0.6us SERIAL descriptor generation per dma_start on the
# SP sequencer.  All loads stay on the SP ring: the Act ring drains much
# more slowly when SP/SWDGE queues are busy (measured), and semaphore-
# paced (consumption-backpressured) loads starve the PE because the
# release->data latency is ~4us (also measured).
XEAGER = {0: (2048, 2048, 4096, 8192), 1: (4096, 12288)}
# y store sizes per head (int8 cols): uniform 2048 keeps the store stream
# smooth (it is paced by eviction throughput).
YSTORES = {0: (2048,) * 8, 1: (2048,) * 8}
# PSUM eviction chunk plan per head: uniform 1024-col chunks (finer
# granularity costs more per-instruction overhead than it saves).
ECHUNKS = {0: (1024,) * 16, 1: (1024,) * 16}
EPS = 1e-5
YCLIP = 4.0                 # int8 y clip point in units of y std (=1)
YSCALE = 127.0 / YCLIP      # device-side PSUM->int8 scale

_CACHE = {}


def _ensure_path():
    for p in ("/opt/trn_rl_repo", "/root/.axon_site/_ro/trn_rl_repo"):
        if os.path.isdir(p) and p not in sys.path:
            sys.path.insert(0, p)
    _install_ntff_hook()


def _install_ntff_hook():
    """The agent image's ``antenv`` lacks ``axon_hooks``, which makes
    ``run_bass_kernel_spmd(trace=True)`` crash instead of degrading.  Provide
    the module and register the ctypes NTFF hook the boot shim would have."""
    if "antenv.axon_hooks" in sys.modules:
        return
    try:
        import types

        import antenv

        if hasattr(antenv, "axon_hooks"):
            return
        mod = types.ModuleType("antenv.axon_hooks")
        state = {"hook": None}
        mod.set_axon_ntff_profile_hook = lambda h: state.__setitem__("hook", h)
        mod.get_axon_ntff_profile_hook = lambda: state["hook"]
        sys.modules["antenv.axon_hooks"] = mod
        antenv.axon_hooks = mod
        try:
            from trn_agent_boot.trn_boot import _ntff_profile_via_ctypes

            so_path = "/opt/axon/libaxon_pjrt.so"
            if os.path.exists(so_path):
                mod.set_axon_ntff_profile_hook(_ntff_profile_via_ctypes(so_path))
        except Exception:
            pass  # hook stays None -> concourse logs + skips tracing
    except Exception:
        pass


def _build_nc():
    """Build the (single-program SPMD) Bass kernel for one core's shard."""
    _ensure_path()
    import concourse.tile as tile
    from concourse import bacc, mybir

    f16 = mybir.dt.float16
    f32 = mybir.dt.float32
    f8 = mybir.dt.float8e3
    i8 = mybir.dt.int8

    nc = bacc.Bacc("TRN2", target_bir_lowering=False, debug=False)
    # x is packed per head as [W^T bytes | x8]: columns 0:WPFX hold the
    # head's fp16 Cayley weight bitcast to fp8 bytes, so the first tile's
    # DMA delivers both W and the first x panel with a single trigger.
    x_d = nc.dram_tensor("xh", [HPC * D, WPFX + T], f8, kind="ExternalInput").ap()
    y_d = nc.dram_tensor("y8", [HPC * D, T], i8, kind="ExternalOutput").ap()

    # PSUM eviction engine rotation (GPSIMD/Pool cannot read PSUM): Act and
    # DVE split 1024-col chunks 17:15 (measured 1.11us vs 1.15us per chunk).
    # Store DMA triggers go on the SP ring behind the loads: SP-ring HWDGE
    # stores measured ~4us faster end-to-end than gpsimd SWDGE stores
    # (lower per-trigger latency and no SWDGE queue teardown).
    def evict_engine(i):
        return "act" if (i * 17) // 32 != ((i - 1) * 17) // 32 else "dve"

    EV = 1024      # eviction chunk (2 PSUM banks per engine instruction)

    with tile.TileContext(nc) as tc:
        with (
            tc.tile_pool(name="xin", bufs=1) as in_pool,
            tc.tile_pool(name="yout", bufs=1) as out_pool,
            tc.tile_pool(name="mmps", bufs=4, space="PSUM") as ps_pool,
        ):
            # --- x DMAs up front (first tile of each head carries the
            # weight); all of x stays resident in SBUF.
            w16s = {}
            xts = {0: [], 1: []}   # (col_start, ap_col_offset, tile)
            for h in range(HPC):
                c0 = 0
                for ti, sz in enumerate(XEAGER[h]):
                    off = WPFX if ti == 0 else 0
                    xt = in_pool.tile([D, off + sz], f8, name=f"x{h}_{ti}",
                                      tag=f"x{h}_{ti}")
                    nc.sync.dma_start(
                        out=xt,
                        in_=x_d[h * D:(h + 1) * D, c0:c0 + off + sz])
                    if ti == 0:
                        w16s[h] = xt[:, 0:WPFX].bitcast(f16)
                    xts[h].append((c0 if ti == 0 else c0 - WPFX, off, xt))
                    c0 += off + sz

            # --- streaming mixed-precision panel matmul: y[h] = W @ x8[h]
            ei = 0
            for h in range(HPC):
                stores = []
                c = 0
                for sz in YSTORES[h]:
                    stores.append((c, sz))
                    c += sz
                chunks = []
                c = 0
                for sz in ECHUNKS[h]:
                    chunks.append((c, sz))
                    c += sz
                si = 0
                ci = 0
                yt = None
                ps = None
                for c0, off, xt in xts[h]:
                    for j in range((xt.shape[-1] - off) // MM):
                        col = c0 + j * MM          # absolute column in head
                        s0, ssz = stores[si]
                        if col == s0:
                            yt = out_pool.tile([D, ssz], i8,
                                               name=f"y{h}_{si}",
                                               tag=f"y{h}_{si}")
                        e0, esz = chunks[ci]
                        if col == e0:
                            ps = ps_pool.tile([D, EV], f32, tag="mm",
                                              name="ps")
                        pc = col - e0
                        nc.tensor.matmul(
                            ps[:, pc:pc + MM], lhsT=w16s[h],
                            rhs=xt[:, off + j * MM:off + (j + 1) * MM],
                            start=True, stop=True)
                        if pc + MM >= esz:         # chunk complete -> evict
                            dst = yt[:, e0 - s0:e0 - s0 + esz]
                            eng = evict_engine(ei)
                            ei += 1
                            if eng == "act":
                                nc.scalar.activation(
                                    dst, ps[:, 0:esz],
                                    mybir.ActivationFunctionType.Copy,
                                    bias=0.0, scale=float(YSCALE))
                            else:
                                nc.vector.tensor_scalar(
                                    dst, ps[:, 0:esz], float(YSCALE), None,
                                    op0=mybir.AluOpType.mult)
                            ci += 1
                        if col + MM == s0 + ssz:
                            nc.sync.dma_start(
                                out=y_d[h * D:(h + 1) * D, s0:s0 + ssz],
                                in_=yt)
                            si += 1
    nc.compile()
    return nc


def _get_nc():
    if "nc" not in _CACHE:
        _CACHE["nc"] = _build_nc()
    return _CACHE["nc"]


def _prep_inputs(x, S_raw):
    """Host-side shard + layout + quantization prep."""
    import ml_dtypes

    x = np.asarray(x, dtype=np.float32)
    S_raw = np.asarray(S_raw, dtype=np.float32)
    S = S_raw - S_raw.transpose(0, 2, 1)
    I = np.eye(D, dtype=np.float32)
    # lhsT for out = lhsT.T @ x  with lhsT.T = W = (I-S) A^{-1}:
    # lhsT = W^T = A^{-T} (I-S)^T = ((1+eps)I - S)^{-1} (I + S)
    WT = np.linalg.solve((1.0 + EPS) * I[None] - S, I[None] + S)  # (H, D, D)
    # fp16 W bytes viewed as fp8 columns (2 bytes per fp16 -> 2*D cols)
    WT8 = WT.astype(np.float16).view(np.uint8).reshape(H, D, WPFX)
    # (B,H,N,D) -> (H, D, B*N), token-major per head, quantized to e3m4
    xT = x.transpose(1, 3, 0, 2).reshape(H, D, T)
    x8 = xT.astype(ml_dtypes.float8_e3m4).view(np.uint8)
    xh = np.ascontiguousarray(
        np.concatenate([WT8, x8], axis=2)).reshape(H * D, WPFX + T)
    in_maps = []
    for c in range(N_CORES):
        r = c * HPC * D
        in_maps.append({"xh": xh[r:r + HPC * D]})
    return in_maps


def _postprocess(results):
    """Gather per-core int8 y shards back into (B, H, N, D) fp32."""
    y8 = np.concatenate([r["y8"] for r in results], axis=0)  # (H*D, T) i8
    y = y8.astype(np.float32) * np.float32(1.0 / YSCALE)
    y = y.reshape(H, D, B, N).transpose(2, 0, 3, 1)
    return np.ascontiguousarray(y)


def _execute(in_maps, trace=False, **kwargs):
    _ensure_path()
    from concourse.bass_utils import run_bass_kernel_spmd

    nc = _get_nc()
    return run_bass_kernel_spmd(nc, in_maps, core_ids=list(range(N_CORES)),
                                trace=trace, **kwargs)


def kernel(x, S_raw):
    in_maps = _prep_inputs(x, S_raw)
    res = _execute(in_maps)
    return _postprocess(res.results)


# revision 41
# speedup vs baseline: 1.0210x; 1.0210x over previous
"""Cayley orthogonal transform kernel for Trainium2 (8 NeuronCores).

Math: per head h, y = (I - S) ((1+eps) I + S)^{-1} x applied along D=128,
where S = S_raw - S_raw^T is skew-symmetric.

Strategy (fp8 in / int8 out over the wire):
  * Host: fold the Cayley weight into a single fp16 matrix per head,
    W^T = ((1+eps)I - S)^{-1} (I + S); lay x out as xT[h, d, token] and
    quantize to fp8 e3m4 (4 mantissa bits, ~1.3% rel L2 for N(0,1) data).
    Heads are sharded 2-per-core across 8 cores (tensor parallel).
  * Device (per core): streaming mixed-precision panel matmul
    psum = W16 @ x8[h] (fp16 stationary x fp8e3 moving runs at full PE
    rate with fp32 accumulate; verified bit-accurate on HW), then each
    1024-col PSUM chunk (2 banks) is requantized to int8 with a single
    global scale (engine float->int casts are round-to-nearest-
    saturating; verified on HW) and stored as int8.  PSUM eviction
    alternates Act/DVE 17:15 (their measured per-chunk times are 1.11
    vs 1.15us; Pool/GPSIMD cannot read PSUM) and is the pipeline's
    steady-state bottleneck at ~1.7 chunks/us; PSUM is 16KB/partition,
    so four 1024-col fp32 tiles is the deepest pipelining possible.
    The fp16 weight rides bitcast inside the first fp8 tile of each
    head, so one DMA delivers both W and the first x panel.  Both load
    and store triggers run on the SP ring (HWDGE): SP-ring stores
    measured ~4us faster than gpsimd SWDGE stores, and the Act ring
    starves under load.  Wire traffic is 1 byte/elem each way (~8.4 MB
    per core vs 16.8 MB for the fp16 baseline), which roughly halves
    the HBM-roofline-bound runtime (measured ~35-36us vs ~52us, of
    which ~7us is a fixed multi-core launch/rendezvous epilogue).
  * Host: dequantize int8 y by the global scale, widen to fp32, inverse
    layout transform back to (B, H, N, D).

  End-to-end rel_l2 vs the fp32 reference = 1.638e-2 (gate: 2e-2,
  deterministic); the error budget is ~1.34% from the e3m4 x
  quantization and ~0.95% from the int8 y requantization, both
  verified against a numpy simulation before the kernel was built.
"""

import os
import sys

import numpy as np

B, H, N, D = 4, 16, 4096, 128
N_CORES = 8
HPC = H // N_CORES          # heads per core
T = B * N                   # tokens per head
MM = 512                    # columns per matmul (one PSUM bank)
WPFX = 2 * D                # fp16 W bitcast into 2*D fp8 columns
# x tile sizes per head (fp8 cols): the first tile of each head carries
# the fp16 weight prefix and stays small so the PE starts early; the
# next tiles stay small enough that each tile's completion semaphore
# fires before the PE reaches its first column (a single 6144-col second
# tile caused a measured 2us PE stall at col 2048); later tiles grow to
# amortize the ~0.6us SERIAL descriptor generation per dma_start on the
# SP sequencer.  All loads stay on the SP ring: the Act ring drains much
# more slowly when SP/SWDGE queues are busy (measured), and semaphore-
# paced (consumption-backpressured) loads starve the PE because the
# release->data latency is ~4us (also measured).
XEAGER = {0: (2048, 2048, 4096, 8192), 1: (4096, 12288)}
# y store sizes per head (int8 cols): uniform 2048 keeps the store stream
# smooth (it is paced by eviction throughput).
YSTORES = {0: (2048,) * 8, 1: (2048,) * 8}
# PSUM eviction chunk plan per head: uniform 1024-col chunks (finer
# granularity costs more per-instruction overhead than it saves).
ECHUNKS = {0: (1024,) * 16, 1: (1024,) * 16}
EPS = 1e-5
YCLIP = 4.0                 # int8 y clip point in units of y std (=1)
YSCALE = 127.0 / YCLIP      # device-side PSUM->int8 scale

_CACHE = {}


def _ensure_path():
    for p in ("/opt/trn_rl_repo", "/root/.axon_site/_ro/trn_rl_repo"):
        if os.path.isdir(p) and p not in sys.path:
            sys.path.insert(0, p)
    _install_ntff_hook()


def _install_ntff_hook():
    """The agent image's ``antenv`` lacks ``axon_hooks``, which makes
    ``run_bass_kernel_spmd(trace=True)`` crash instead of degrading.  Provide
    the module and register the ctypes NTFF hook the boot shim would have."""
    if "antenv.axon_hooks" in sys.modules:
        return
    try:
        import types

        import antenv

        if hasattr(antenv, "axon_hooks"):
            return
        mod = types.ModuleType("antenv.axon_hooks")
        state = {"hook": None}
        mod.set_axon_ntff_profile_hook = lambda h: state.__setitem__("hook", h)
        mod.get_axon_ntff_profile_hook = lambda: state["hook"]
        sys.modules["antenv.axon_hooks"] = mod
        antenv.axon_hooks = mod
        try:
            from trn_agent_boot.trn_boot import _ntff_profile_via_ctypes

            so_path = "/opt/axon/libaxon_pjrt.so"
            if os.path.exists(so_path):
                mod.set_axon_ntff_profile_hook(_ntff_profile_via_ctypes(so_path))
        except Exception:
            pass  # hook stays None -> concourse logs + skips tracing
    except Exception:
        pass


def _build_nc():
    """Build the (single-program SPMD) Bass kernel for one core's shard."""
    _ensure_path()
    import concourse.tile as tile
    from concourse import bacc, mybir

    f16 = mybir.dt.float16
    f32 = mybir.dt.float32
    f8 = mybir.dt.float8e3
    i8 = mybir.dt.int8

    nc = bacc.Bacc("TRN2", target_bir_lowering=False, debug=False)
    # x is packed per head as [W^T bytes | x8]: columns 0:WPFX hold the
    # head's fp16 Cayley weight bitcast to fp8 bytes, so the first tile's
    # DMA delivers both W and the first x panel with a single trigger.
    x_d = nc.dram_tensor("xh", [HPC * D, WPFX + T], f8, kind="ExternalInput").ap()
    y_d = nc.dram_tensor("y8", [HPC * D, T], i8, kind="ExternalOutput").ap()

    # PSUM eviction engine rotation (GPSIMD/Pool cannot read PSUM): Act and
    # DVE split 1024-col chunks 17:15 (measured 1.11us vs 1.15us per chunk).
    # Store DMA triggers go on the SP ring behind the loads: SP-ring HWDGE
    # stores measured ~4us faster end-to-end than gpsimd SWDGE stores
    # (lower per-trigger latency and no SWDGE queue teardown).
    def evict_engine(i):
        return "act" if (i * 17) // 32 != ((i - 1) * 17) // 32 else "dve"

    EV = 1024      # eviction chunk (2 PSUM banks per engine instruction)

    with tile.TileContext(nc) as tc:
        with (
            tc.tile_pool(name="xin", bufs=1) as in_pool,
            tc.tile_pool(name="yout", bufs=1) as out_pool,
            tc.tile_pool(name="mmps", bufs=4, space="PSUM") as ps_pool,
        ):
            # --- x DMAs up front (first tile of each head carries the
            # weight); all of x stays resident in SBUF.
            w16s = {}
            xts = {0: [], 1: []}   # (col_start, ap_col_offset, tile)
            for h in range(HPC):
                c0 = 0
                for ti, sz in enumerate(XEAGER[h]):
                    off = WPFX if ti == 0 else 0
                    xt = in_pool.tile([D, off + sz], f8, name=f"x{h}_{ti}",
                                      tag=f"x{h}_{ti}")
                    # head 0's first tile goes on the (empty-at-launch) Act
                    # ring so its descriptor gen runs in parallel with SP
                    # generating tile 2 -> the PE starts ~1us earlier.
                    trig = nc.scalar if (h == 0 and ti == 0) else nc.sync
                    trig.dma_start(
                        out=xt,
                        in_=x_d[h * D:(h + 1) * D, c0:c0 + off + sz])
                    if ti == 0:
                        w16s[h] = xt[:, 0:WPFX].bitcast(f16)
                    xts[h].append((c0 if ti == 0 else c0 - WPFX, off, xt))
                    c0 += off + sz

            # --- streaming mixed-precision panel matmul: y[h] = W @ x8[h]
            ei = 0
            for h in range(HPC):
                stores = []
                c = 0
                for sz in YSTORES[h]:
                    stores.append((c, sz))
                    c += sz
                chunks = []
                c = 0
                for sz in ECHUNKS[h]:
                    chunks.append((c, sz))
                    c += sz
                si = 0
                ci = 0
                yt = None
                ps = None
                for c0, off, xt in xts[h]:
                    for j in range((xt.shape[-1] - off) // MM):
                        col = c0 + j * MM          # absolute column in head
                        s0, ssz = stores[si]
                        if col == s0:
                            yt = out_pool.tile([D, ssz], i8,
                                               name=f"y{h}_{si}",
                                               tag=f"y{h}_{si}")
                        e0, esz = chunks[ci]
                        if col == e0:
                            ps = ps_pool.tile([D, EV], f32, tag="mm",
                                              name="ps")
                        pc = col - e0
                        nc.tensor.matmul(
                            ps[:, pc:pc + MM], lhsT=w16s[h],
                            rhs=xt[:, off + j * MM:off + (j + 1) * MM],
                            start=True, stop=True)
                        if pc + MM >= esz:         # chunk complete -> evict
                            dst = yt[:, e0 - s0:e0 - s0 + esz]
                            eng = evict_engine(ei)
                            ei += 1
                            if eng == "act":
                                nc.scalar.activation(
                                    dst, ps[:, 0:esz],
                                    mybir.ActivationFunctionType.Copy,
                                    bias=0.0, scale=float(YSCALE))
                            else:
                                nc.vector.tensor_scalar(
                                    dst, ps[:, 0:esz], float(YSCALE), None,
                                    op0=mybir.AluOpType.mult)
                            ci += 1
                        if col + MM == s0 + ssz:
                            nc.sync.dma_start(
                                out=y_d[h * D:(h + 1) * D, s0:s0 + ssz],
                                in_=yt)
                            si += 1
    nc.compile()
    return nc


def _get_nc():
    if "nc" not in _CACHE:
        _CACHE["nc"] = _build_nc()
    return _CACHE["nc"]


def _prep_inputs(x, S_raw):
    """Host-side shard + layout + quantization prep."""
    import ml_dtypes

    x = np.asarray(x, dtype=np.float32)
    S_raw = np.asarray(S_raw, dtype=np.float32)
    S = S_raw - S_raw.transpose(0, 2, 1)
    I = np.eye(D, dtype=np.float32)
    # lhsT for out = lhsT.T @ x  with lhsT.T = W = (I-S) A^{-1}:
    # lhsT = W^T = A^{-T} (I-S)^T = ((1+eps)I - S)^{-1} (I + S)
    WT = np.linalg.solve((1.0 + EPS) * I[None] - S, I[None] + S)  # (H, D, D)
    # fp16 W bytes viewed as fp8 columns (2 bytes per fp16 -> 2*D cols)
    WT8 = WT.astype(np.float16).view(np.uint8).reshape(H, D, WPFX)
    # (B,H,N,D) -> (H, D, B*N), token-major per head, quantized to e3m4
    xT = x.transpose(1, 3, 0, 2).reshape(H, D, T)
    x8 = xT.astype(ml_dtypes.float8_e3m4).view(np.uint8)
    xh = np.ascontiguousarray(
        np.concatenate([WT8, x8], axis=2)).reshape(H * D, WPFX + T)
    in_maps = []
    for c in range(N_CORES):
        r = c * HPC * D
        in_maps.append({"xh": xh[r:r + HPC * D]})
    return in_maps


def _postprocess(results):
    """Gather per-core int8 y shards back into (B, H, N, D) fp32."""
    y8 = np.concatenate([r["y8"] for r in results], axis=0)  # (H*D, T) i8
    y = y8.astype(np.float32) * np.float32(1.0 / YSCALE)
    y = y.reshape(H, D, B, N).transpose(2, 0, 3, 1)
    return np.ascontiguousarray(y)


def _execute(in_maps, trace=False, **kwargs):
    _ensure_path()
    from concourse.bass_utils import run_bass_kernel_spmd

    nc = _get_nc()
    return run_bass_kernel_spmd(nc, in_maps, core_ids=list(range(N_CORES)),
                                trace=trace, **kwargs)


def kernel(x, S_raw):
    in_maps = _prep_inputs(x, S_raw)
    res = _execute(in_maps)
    return _postprocess(res.results)


# revision 42
# speedup vs baseline: 1.1177x; 1.0947x over previous
"""Cayley orthogonal transform kernel for Trainium2 (8 NeuronCores).

Math: per head h, y = (I - S) ((1+eps) I + S)^{-1} x applied along D=128,
where S = S_raw - S_raw^T is skew-symmetric.

Strategy (fp8 in / int8 out over the wire):
  * Host: fold the Cayley weight into a single fp16 matrix per head,
    W^T = ((1+eps)I - S)^{-1} (I + S); lay x out as xT[h, d, token] and
    quantize to fp8 e3m4 (4 mantissa bits, ~1.3% rel L2 for N(0,1) data).
    Heads are sharded 2-per-core across 8 cores (tensor parallel).
  * Device (per core): streaming mixed-precision panel matmul
    psum = W16 @ x8[h] (fp16 stationary x fp8e3 moving runs at full PE
    rate with fp32 accumulate; verified bit-accurate on HW), then each
    1024-col PSUM chunk (2 banks) is requantized to int8 with a single
    global scale (engine float->int casts are round-to-nearest-
    saturating; verified on HW) and stored as int8.  PSUM eviction
    alternates Act/DVE 17:15 (their measured per-chunk times are 1.11
    vs 1.15us; Pool/GPSIMD cannot read PSUM) and is the pipeline's
    steady-state bottleneck at ~1.7 chunks/us; PSUM is 16KB/partition,
    so four 1024-col fp32 tiles is the deepest pipelining possible.
    The fp16 weight rides bitcast inside the first fp8 tile of each
    head, so one DMA delivers both W and the first x panel.  Both load
    and store triggers run on the SP ring (HWDGE): SP-ring stores
    measured ~4us faster than gpsimd SWDGE stores, and the Act ring
    starves under load.  Wire traffic is 1 byte/elem each way (~8.4 MB
    per core vs 16.8 MB for the fp16 baseline), which roughly halves
    the HBM-roofline-bound runtime (measured ~35-36us vs ~52us, of
    which ~7us is a fixed multi-core launch/rendezvous epilogue).
  * Host: dequantize int8 y by the global scale, widen to fp32, inverse
    layout transform back to (B, H, N, D).

  End-to-end rel_l2 vs the fp32 reference = 1.638e-2 (gate: 2e-2,
  deterministic); the error budget is ~1.34% from the e3m4 x
  quantization and ~0.95% from the int8 y requantization, both
  verified against a numpy simulation before the kernel was built.
"""

import os
import sys

import numpy as np

B, H, N, D = 4, 16, 4096, 128
N_CORES = 8
HPC = H // N_CORES          # heads per core
T = B * N                   # tokens per head
MM = 512                    # columns per matmul (one PSUM bank)
WPFX = 2 * D                # fp16 W bitcast into 2*D fp8 columns
# x tile sizes per head (fp8 cols): the first tile of each head carries
# the fp16 weight prefix and stays small so the PE starts early; the
# next tiles stay small enough that each tile's completion semaphore
# fires before the PE reaches its first column (a single 6144-col second
# tile caused a measured 2us PE stall at col 2048); later tiles grow to
# amortize the ~0.6us SERIAL descriptor generation per dma_start on the
# SP sequencer.  All loads stay on the SP ring: the Act ring drains much
# more slowly when SP/SWDGE queues are busy (measured), and semaphore-
# paced (consumption-backpressured) loads starve the PE because the
# release->data latency is ~4us (also measured).
XEAGER = {0: (2048, 2048, 4096, 4096, 4096), 1: (4096, 12288)}
# y store sizes per head (int8 cols): uniform 2048 keeps the store stream
# smooth (it is paced by eviction throughput).
YSTORES = {0: (2048,) * 8, 1: (2048,) * 8}
# PSUM eviction chunk plan per head: uniform 1024-col chunks (finer
# granularity costs more per-instruction overhead than it saves).
ECHUNKS = {0: (1024,) * 16, 1: (1024,) * 16}
EPS = 1e-5
YCLIP = 4.0                 # int8 y clip point in units of y std (=1)
YSCALE = 127.0 / YCLIP      # device-side PSUM->int8 scale

_CACHE = {}


def _ensure_path():
    for p in ("/opt/trn_rl_repo", "/root/.axon_site/_ro/trn_rl_repo"):
        if os.path.isdir(p) and p not in sys.path:
            sys.path.insert(0, p)
    _install_ntff_hook()


def _install_ntff_hook():
    """The agent image's ``antenv`` lacks ``axon_hooks``, which makes
    ``run_bass_kernel_spmd(trace=True)`` crash instead of degrading.  Provide
    the module and register the ctypes NTFF hook the boot shim would have."""
    if "antenv.axon_hooks" in sys.modules:
        return
    try:
        import types

        import antenv

        if hasattr(antenv, "axon_hooks"):
            return
        mod = types.ModuleType("antenv.axon_hooks")
        state = {"hook": None}
        mod.set_axon_ntff_profile_hook = lambda h: state.__setitem__("hook", h)
        mod.get_axon_ntff_profile_hook = lambda: state["hook"]
        sys.modules["antenv.axon_hooks"] = mod
        antenv.axon_hooks = mod
        try:
            from trn_agent_boot.trn_boot import _ntff_profile_via_ctypes

            so_path = "/opt/axon/libaxon_pjrt.so"
            if os.path.exists(so_path):
                mod.set_axon_ntff_profile_hook(_ntff_profile_via_ctypes(so_path))
        except Exception:
            pass  # hook stays None -> concourse logs + skips tracing
    except Exception:
        pass


def _build_nc():
    """Build the (single-program SPMD) Bass kernel for one core's shard."""
    _ensure_path()
    import concourse.tile as tile
    from concourse import bacc, mybir

    f16 = mybir.dt.float16
    f32 = mybir.dt.float32
    f8 = mybir.dt.float8e3
    i8 = mybir.dt.int8

    nc = bacc.Bacc("TRN2", target_bir_lowering=False, debug=False)
    # x is packed per head as [W^T bytes | x8]: columns 0:WPFX hold the
    # head's fp16 Cayley weight bitcast to fp8 bytes, so the first tile's
    # DMA delivers both W and the first x panel with a single trigger.
    x_d = nc.dram_tensor("xh", [HPC * D, WPFX + T], f8, kind="ExternalInput").ap()
    y_d = nc.dram_tensor("y8", [HPC * D, T], i8, kind="ExternalOutput").ap()

    # PSUM eviction engine rotation (GPSIMD/Pool cannot read PSUM): Act and
    # DVE split 1024-col chunks 17:15 (measured 1.11us vs 1.15us per chunk).
    # Store DMA triggers go on the SP ring behind the loads: SP-ring HWDGE
    # stores measured ~4us faster end-to-end than gpsimd SWDGE stores
    # (lower per-trigger latency and no SWDGE queue teardown).
    def evict_engine(i):
        return "act" if (i * 17) // 32 != ((i - 1) * 17) // 32 else "dve"

    EV = 1024      # eviction chunk (2 PSUM banks per engine instruction)

    with tile.TileContext(nc) as tc:
        with (
            tc.tile_pool(name="xin", bufs=1) as in_pool,
            tc.tile_pool(name="yout", bufs=1) as out_pool,
            tc.tile_pool(name="mmps", bufs=4, space="PSUM") as ps_pool,
        ):
            # --- x DMAs up front (first tile of each head carries the
            # weight); all of x stays resident in SBUF.
            w16s = {}
            xts = {0: [], 1: []}   # (col_start, ap_col_offset, tile)
            for h in range(HPC):
                c0 = 0
                for ti, sz in enumerate(XEAGER[h]):
                    off = WPFX if ti == 0 else 0
                    xt = in_pool.tile([D, off + sz], f8, name=f"x{h}_{ti}",
                                      tag=f"x{h}_{ti}")
                    nc.sync.dma_start(
                        out=xt,
                        in_=x_d[h * D:(h + 1) * D, c0:c0 + off + sz])
                    if ti == 0:
                        w16s[h] = xt[:, 0:WPFX].bitcast(f16)
                    xts[h].append((c0 if ti == 0 else c0 - WPFX, off, xt))
                    c0 += off + sz

            # --- streaming mixed-precision panel matmul: y[h] = W @ x8[h]
            ei = 0
            for h in range(HPC):
                stores = []
                c = 0
                for sz in YSTORES[h]:
                    stores.append((c, sz))
                    c += sz
                chunks = []
                c = 0
                for sz in ECHUNKS[h]:
                    chunks.append((c, sz))
                    c += sz
                si = 0
                ci = 0
                yt = None
                ps = None
                for c0, off, xt in xts[h]:
                    for j in range((xt.shape[-1] - off) // MM):
                        col = c0 + j * MM          # absolute column in head
                        s0, ssz = stores[si]
                        if col == s0:
                            yt = out_pool.tile([D, ssz], i8,
                                               name=f"y{h}_{si}",
                                               tag=f"y{h}_{si}")
                        e0, esz = chunks[ci]
                        if col == e0:
                            ps = ps_pool.tile([D, EV], f32, tag="mm",
                                              name="ps")
                        pc = col - e0
                        nc.tensor.matmul(
                            ps[:, pc:pc + MM], lhsT=w16s[h],
                            rhs=xt[:, off + j * MM:off + (j + 1) * MM],
                            start=True, stop=True)
                        if pc + MM >= esz:         # chunk complete -> evict
                            dst = yt[:, e0 - s0:e0 - s0 + esz]
                            eng = evict_engine(ei)
                            ei += 1
                            if eng == "act":
                                nc.scalar.activation(
                                    dst, ps[:, 0:esz],
                                    mybir.ActivationFunctionType.Copy,
                                    bias=0.0, scale=float(YSCALE))
                            else:
                                nc.vector.tensor_scalar(
                                    dst, ps[:, 0:esz], float(YSCALE), None,
                                    op0=mybir.AluOpType.mult)
                            ci += 1
                        if col + MM == s0 + ssz:
                            nc.sync.dma_start(
                                out=y_d[h * D:(h + 1) * D, s0:s0 + ssz],
                                in_=yt)
                            si += 1
    nc.compile()
    return nc


def _get_nc():
    if "nc" not in _CACHE:
        _CACHE["nc"] = _build_nc()
    return _CACHE["nc"]


def _prep_inputs(x, S_raw):
    """Host-side shard + layout + quantization prep."""
    import ml_dtypes

    x = np.asarray(x, dtype=np.float32)
    S_raw = np.asarray(S_raw, dtype=np.float32)
    S = S_raw - S_raw.transpose(0, 2, 1)
    I = np.eye(D, dtype=np.float32)
    # lhsT for out = lhsT.T @ x  with lhsT.T = W = (I-S) A^{-1}:
    # lhsT = W^T = A^{-T} (I-S)^T = ((1+eps)I - S)^{-1} (I + S)
    WT = np.linalg.solve((1.0 + EPS) * I[None] - S, I[None] + S)  # (H, D, D)
    # fp16 W bytes viewed as fp8 columns (2 bytes per fp16 -> 2*D cols)
    WT8 = WT.astype(np.float16).view(np.uint8).reshape(H, D, WPFX)
    # (B,H,N,D) -> (H, D, B*N), token-major per head, quantized to e3m4
    xT = x.transpose(1, 3, 0, 2).reshape(H, D, T)
    x8 = xT.astype(ml_dtypes.float8_e3m4).view(np.uint8)
    xh = np.ascontiguousarray(
        np.concatenate([WT8, x8], axis=2)).reshape(H * D, WPFX + T)
    in_maps = []
    for c in range(N_CORES):
        r = c * HPC * D
        in_maps.append({"xh": xh[r:r + HPC * D]})
    return in_maps


def _postprocess(results):
    """Gather per-core int8 y shards back into (B, H, N, D) fp32."""
    y8 = np.concatenate([r["y8"] for r in results], axis=0)  # (H*D, T) i8
    y = y8.astype(np.float32) * np.float32(1.0 / YSCALE)
    y = y.reshape(H, D, B, N).transpose(2, 0, 3, 1)
    return np.ascontiguousarray(y)


def _execute(in_maps, trace=False, **kwargs):
    _ensure_path()
    from concourse.bass_utils import run_bass_kernel_spmd

    nc = _get_nc()
    return run_bass_kernel_spmd(nc, in_maps, core_ids=list(range(N_CORES)),
                                trace=trace, **kwargs)


def kernel(x, S_raw):
    in_maps = _prep_inputs(x, S_raw)
    res = _execute(in_maps)
    return _postprocess(res.results)
